# revision 22
# baseline (speedup 1.0000x reference)
"""Trainium2 Bass kernel for ConditionalThetaDiagonalSplineLinearXFlowMLP.

Computes out = (phi(theta) @ Wa.T + ca) * x + (phi(theta) @ Wb.T + cb)
where phi is the cubic B-spline basis (5 functions, knots [0,0,0,0,.5,1,1,1,1]).

Sharding: pure data parallel over the batch axis across 8 cores; the tiny
spline params are replicated.

The stream is HBM-bound (~64 MB/core in f32).  To cut DMA bytes, the kernel
runs the spline/matmul pipeline in f16: x is cast f32->f16 during the load
DMA (SWDGE), phi/weights are f16, and the output is written as f16 and
widened to f32 on the host during the unshard.  End-to-end rounding is
~1e-3 relative, far inside the 2e-2 gate, and halves both DMA streams.

Device-side algorithm per core (B_SHARD=2048 rows):
  1. phi computed on DVE as two f16 Horner passes (lo/hi segment piecewise
     cubics with per-partition coefficients on partitions 0..5) + predicated
     select on u>=0.5, chunked so the first tile's matmuls start early.
     Partition row 5 carries coefficient (0,0,0,1) so the Horner itself
     produces the constant 1.0 bias row of the stationary operand.
  2. Per 128-row tile, per 1024-col chunk: K=6 f16 matmuls compute
     a=phi6^T@[Wa^T;ca] into PSUM (start=True), DVE multiplies PSUM in place
     by x, the b matmuls accumulate on top (start=False), ScalarE copies
     PSUM -> SBUF with an f32->f16 cast, HWDGE DMA writes out.
"""

import numpy as np

import concourse.bass as bass
from concourse import bacc
import concourse.mybir as mybir
from concourse.bass_utils import run_bass_kernel_spmd
from concourse.tile import TileContext

F32 = mybir.dt.float32
F16 = mybir.dt.float16
ALU = mybir.AluOpType

N_CORES = 8
B, D, K = 16384, 4096, 5
B_SHARD = B // N_CORES          # 2048
P = 128                          # partitions per row tile
N_TILES = B_SHARD // P           # 16
CHUNK = 2048                     # psum chunk columns (4 banks)
MM_N = 512                       # matmul moving free dim (psum bank pair)
PSUM_BUFS = 2                    # 2 x 4 banks = all 8 banks
PHI_CHUNK = 1024                 # phi computed in B_SHARD/PHI_CHUNK pieces

# Piecewise-cubic coefficients of the 5 basis functions, phi = A u^3 + B u^2
# + C u + D, derived exactly from the clamped knot vector [0,0,0,0,.5,1,1,1,1].
# Rows: basis k = 0..4. Columns: A,B,C,D for u in [0,.5) then A,B,C,D for
# u in [.5,1).  All values are exactly representable in f16.
SPLINE_COEF = np.array(
    [
        [-8.0, 12.0, -6.0, 1.0,   0.0, 0.0, 0.0, 0.0],
        [14.0, -18.0, 6.0, 0.0,  -2.0, 6.0, -6.0, 2.0],
        [-8.0, 6.0, 0.0, 0.0,     8.0, -18.0, 12.0, -2.0],
        [2.0, 0.0, 0.0, 0.0,    -14.0, 24.0, -12.0, 2.0],
        [0.0, 0.0, 0.0, 0.0,      8.0, -12.0, 6.0, -1.0],
    ],
    dtype=np.float32,
)

U_LO = 1e-6
U_HI = 1.0 - 1e-6


def _build_nc():
    nc = bacc.Bacc("TRN2")
    x = nc.dram_tensor("x", [B_SHARD, D], F32, kind="ExternalInput")
    # thetab: theta broadcast on K+1 partitions (f16); coefb: the per-basis
    # piecewise Horner coefficients (f32 — DVE scalar operands must be f32).
    thetab = nc.dram_tensor("thetab", [K + 1, B_SHARD], F16, kind="ExternalInput")
    coefb = nc.dram_tensor("coefb", [K + 1, 8], F32, kind="ExternalInput")
    # wab: compact stationary weights [6, 2D]: cols 0:D = [Wa.T; ca],
    # cols D:2D = [Wb.T; cb].
    wab = nc.dram_tensor("wab", [K + 1, 2 * D], F16, kind="ExternalInput")
    out = nc.dram_tensor("out", [B_SHARD, D], F16, kind="ExternalOutput")

    with TileContext(nc) as tc:
        with (
            tc.tile_pool(name="const", bufs=1) as cpool,
            tc.tile_pool(name="xp", bufs=10) as xpool,
            tc.tile_pool(name="op", bufs=4) as opool,
            tc.tile_pool(name="pp", bufs=PSUM_BUFS, space="PSUM") as ppool,
        ):
            # ---- constant loads ----
            theta_sb = cpool.tile([K + 1, B_SHARD], F16)
            nc.sync.dma_start(out=theta_sb, in_=thetab[:, :])
            coef_sb = cpool.tile([K + 1, 8], F32)
            nc.sync.dma_start(out=coef_sb, in_=coefb[:, :])
            w_sb = cpool.tile([K + 1, 2 * D], F16)
            nc.sync.dma_start(out=w_sb, in_=wab[:, :])

            # ---- phi on DVE: [K+1, B_SHARD] f16, partitions 0..5
            phi6 = cpool.tile([K + 1, B_SHARD], F16)
            u = cpool.tile([K + 1, B_SHARD], F16)
            phi_hi = cpool.tile([K + 1, B_SHARD], F16)

            def cf(j):
                return coef_sb[:, j : j + 1]

            def emit_phi_chunk(pc):
                """Horner for phi columns [pc*PHI_CHUNK, (pc+1)*PHI_CHUNK)."""
                cols = slice(pc * PHI_CHUNK, (pc + 1) * PHI_CHUNK)
                ut = u[:, cols]
                lo = phi6[:, cols]
                hi = phi_hi[:, cols]
                # u = clip(theta, 1e-6, 1-1e-6) (equivalent to the reference's
                # clip(clip(theta,0,1), 1e-6, 1-1e-6) up to f16 rounding)
                nc.vector.tensor_scalar(
                    ut, theta_sb[:, cols], U_LO, U_HI, ALU.max, ALU.min
                )
                # Horner: ((A*u + B)*u + C)*u + D with per-partition A..D
                nc.vector.tensor_scalar(lo, ut, cf(0), None, ALU.mult)
                nc.vector.scalar_tensor_tensor(lo, lo, cf(1), ut, ALU.add, ALU.mult)
                nc.vector.scalar_tensor_tensor(lo, lo, cf(2), ut, ALU.add, ALU.mult)
                nc.vector.tensor_scalar(lo, lo, cf(3), None, ALU.add)

                nc.vector.tensor_scalar(hi, ut, cf(4), None, ALU.mult)
                nc.vector.scalar_tensor_tensor(hi, hi, cf(5), ut, ALU.add, ALU.mult)
                nc.vector.scalar_tensor_tensor(hi, hi, cf(6), ut, ALU.add, ALU.mult)
                nc.vector.tensor_scalar(hi, hi, cf(7), None, ALU.add)

                # mask overwrites u (no longer needed); CopyPredicated wants an
                # integer mask; bitcast f16 1.0/0.0 (0x3C00/0x0).
                nc.vector.tensor_scalar(ut, ut, 0.5, None, ALU.is_ge)
                nc.vector.copy_predicated(lo, ut.bitcast(mybir.dt.uint16), hi)

            # ---- main streaming loop ----
            # phi chunks are emitted just-in-time inside the tile loop so the
            # in-order DVE queue doesn't stall tile 0 behind all of phi.
            tiles_per_phi = PHI_CHUNK // P
            for j in range(N_TILES):
                if j % tiles_per_phi == 0:
                    emit_phi_chunk(j // tiles_per_phi)
                rows = slice(j * P, (j + 1) * P)
                # f32 -> f16 cast during the DMA (SWDGE): halves the SBUF-side
                # bytes of the dominant x stream; ~5e-4 relative rounding on x.
                xt = xpool.tile([P, D], F16, tag="xt")
                nc.gpsimd.dma_start(out=xt, in_=x[rows, :])
                ot = opool.tile([P, D], F16)

                for c in range(D // CHUNK):
                    cols = slice(c * CHUNK, (c + 1) * CHUNK)
                    ps = ppool.tile([P, CHUNK], F32)
                    for s in range(CHUNK // MM_N):
                        nc.tensor.matmul(
                            ps[:, s * MM_N : (s + 1) * MM_N],
                            phi6[:, j * P : (j + 1) * P],
                            w_sb[:, c * CHUNK + s * MM_N : c * CHUNK + (s + 1) * MM_N],
                            start=True,
                            stop=False,
                        )
                    nc.vector.tensor_mul(out=ps, in0=ps, in1=xt[:, cols])
                    for s in range(CHUNK // MM_N):
                        nc.tensor.matmul(
                            ps[:, s * MM_N : (s + 1) * MM_N],
                            phi6[:, j * P : (j + 1) * P],
                            w_sb[
                                :,
                                D + c * CHUNK + s * MM_N : D
                                + c * CHUNK
                                + (s + 1) * MM_N,
                            ],
                            start=False,
                            stop=True,
                        )
                    # PSUM -> SBUF with f32 -> f16 cast
                    nc.scalar.copy(out=ot[:, cols], in_=ps)
                nc.scalar.dma_start(out=out[rows, :], in_=ot)
    nc.compile()
    return nc


_NC_CACHE = None


def _get_nc():
    global _NC_CACHE
    if _NC_CACHE is None:
        _NC_CACHE = _build_nc()
    return _NC_CACHE


def _make_in_maps(x, theta, Wa, ca, Wb, cb):
    x = np.ascontiguousarray(x, dtype=np.float32)
    theta = np.ascontiguousarray(theta, dtype=np.float32).reshape(-1)
    wab = np.empty((K + 1, 2 * D), dtype=np.float16)
    wab[:K, :D] = Wa.T.astype(np.float16)
    wab[K, :D] = ca.astype(np.float16)
    wab[:K, D:] = Wb.T.astype(np.float16)
    wab[K, D:] = cb.astype(np.float16)
    coef6 = np.zeros((K + 1, 8), dtype=np.float32)
    coef6[:K] = SPLINE_COEF
    coef6[K] = [0, 0, 0, 1, 0, 0, 0, 1]  # bias row: poly == 1.0
    in_maps = []
    for core in range(N_CORES):
        rows = slice(core * B_SHARD, (core + 1) * B_SHARD)
        thetab = np.broadcast_to(
            theta[rows][None, :].astype(np.float16), (K + 1, B_SHARD)
        ).copy()
        in_maps.append(
            {
                "x": np.ascontiguousarray(x[rows]),
                "thetab": thetab,
                "coefb": coef6,
                "wab": wab,
            }
        )
    return in_maps


def _run(inputs, trace=False, **kwargs):
    nc = _get_nc()
    in_maps = _make_in_maps(**inputs)
    res = run_bass_kernel_spmd(
        nc, in_maps, core_ids=list(range(N_CORES)), trace=trace, **kwargs
    )
    # Device computes/stores f16; widen to the reference's f32 during unshard.
    out = np.concatenate([r["out"] for r in res.results], axis=0).astype(np.float32)
    return out, res


def kernel(**inputs):
    out, _ = _run(inputs, trace=False)
    return out


# revision 23
# speedup vs baseline: 1.0763x; 1.0763x over previous
"""Trainium2 Bass kernel for ConditionalThetaDiagonalSplineLinearXFlowMLP.

Computes out = (phi(theta) @ Wa.T + ca) * x + (phi(theta) @ Wb.T + cb)
where phi is the cubic B-spline basis (5 functions, knots [0,0,0,0,.5,1,1,1,1]).

Sharding: pure data parallel over the batch axis across 8 cores; the tiny
spline params are replicated.

The stream is HBM-bound (~64 MB/core in f32).  To cut DMA bytes, the kernel
runs the spline/matmul pipeline in f16: x is cast f32->f16 during the load
DMA (SWDGE), phi/weights are f16, and the output is written as f16 and
widened to f32 on the host during the unshard.  End-to-end rounding is
~1e-3 relative, far inside the 2e-2 gate, and halves both DMA streams.

Device-side algorithm per core (B_SHARD=2048 rows):
  1. phi computed on DVE as two f16 Horner passes (lo/hi segment piecewise
     cubics with per-partition coefficients on partitions 0..5) + predicated
     select on u>=0.5, chunked so the first tile's matmuls start early.
     Partition row 5 carries coefficient (0,0,0,1) so the Horner itself
     produces the constant 1.0 bias row of the stationary operand.
  2. Per 128-row tile, per 1024-col chunk: K=6 f16 matmuls compute
     a=phi6^T@[Wa^T;ca] into PSUM (start=True), DVE multiplies PSUM in place
     by x, the b matmuls accumulate on top (start=False), ScalarE copies
     PSUM -> SBUF with an f32->f16 cast, HWDGE DMA writes out.
"""

import numpy as np

import concourse.bass as bass
from concourse import bacc
import concourse.mybir as mybir
from concourse.bass_utils import run_bass_kernel_spmd
from concourse.tile import TileContext

F32 = mybir.dt.float32
F16 = mybir.dt.float16
ALU = mybir.AluOpType

N_CORES = 8
B, D, K = 16384, 4096, 5
B_SHARD = B // N_CORES          # 2048
P = 128                          # partitions per row tile
N_TILES = B_SHARD // P           # 16
CHUNK = 1024                     # psum chunk columns (2 banks)
MM_N = 512                       # matmul moving free dim (psum bank pair)
PSUM_BUFS = 4                    # 4 x 2 banks = all 8 banks
PHI_CHUNK = 1024                 # phi computed in B_SHARD/PHI_CHUNK pieces

# Piecewise-cubic coefficients of the 5 basis functions, phi = A u^3 + B u^2
# + C u + D, derived exactly from the clamped knot vector [0,0,0,0,.5,1,1,1,1].
# Rows: basis k = 0..4. Columns: A,B,C,D for u in [0,.5) then A,B,C,D for
# u in [.5,1).  All values are exactly representable in f16.
SPLINE_COEF = np.array(
    [
        [-8.0, 12.0, -6.0, 1.0,   0.0, 0.0, 0.0, 0.0],
        [14.0, -18.0, 6.0, 0.0,  -2.0, 6.0, -6.0, 2.0],
        [-8.0, 6.0, 0.0, 0.0,     8.0, -18.0, 12.0, -2.0],
        [2.0, 0.0, 0.0, 0.0,    -14.0, 24.0, -12.0, 2.0],
        [0.0, 0.0, 0.0, 0.0,      8.0, -12.0, 6.0, -1.0],
    ],
    dtype=np.float32,
)

U_LO = 1e-6
U_HI = 1.0 - 1e-6


def _build_nc():
    nc = bacc.Bacc("TRN2")
    x = nc.dram_tensor("x", [B_SHARD, D], F32, kind="ExternalInput")
    # thetab: theta broadcast on K+1 partitions (f16); coefb: the per-basis
    # piecewise Horner coefficients (f32 — DVE scalar operands must be f32).
    thetab = nc.dram_tensor("thetab", [K + 1, B_SHARD], F16, kind="ExternalInput")
    coefb = nc.dram_tensor("coefb", [K + 1, 8], F32, kind="ExternalInput")
    # wab: compact stationary weights [6, 2D]: cols 0:D = [Wa.T; ca],
    # cols D:2D = [Wb.T; cb].
    wab = nc.dram_tensor("wab", [K + 1, 2 * D], F16, kind="ExternalInput")
    out = nc.dram_tensor("out", [B_SHARD, D], F16, kind="ExternalOutput")

    with TileContext(nc) as tc:
        with (
            tc.tile_pool(name="const", bufs=1) as cpool,
            tc.tile_pool(name="xp", bufs=10) as xpool,
            tc.tile_pool(name="op", bufs=4) as opool,
            tc.tile_pool(name="pp", bufs=PSUM_BUFS, space="PSUM") as ppool,
        ):
            # ---- constant loads ----
            theta_sb = cpool.tile([K + 1, B_SHARD], F16)
            nc.sync.dma_start(out=theta_sb, in_=thetab[:, :])
            coef_sb = cpool.tile([K + 1, 8], F32)
            nc.sync.dma_start(out=coef_sb, in_=coefb[:, :])
            w_sb = cpool.tile([K + 1, 2 * D], F16)
            nc.sync.dma_start(out=w_sb, in_=wab[:, :])

            # ---- phi on DVE: [K+1, B_SHARD] f16, partitions 0..5
            phi6 = cpool.tile([K + 1, B_SHARD], F16)
            u = cpool.tile([K + 1, B_SHARD], F16)
            phi_hi = cpool.tile([K + 1, B_SHARD], F16)

            def cf(j):
                return coef_sb[:, j : j + 1]

            def emit_phi_chunk(pc):
                """Horner for phi columns [pc*PHI_CHUNK, (pc+1)*PHI_CHUNK)."""
                cols = slice(pc * PHI_CHUNK, (pc + 1) * PHI_CHUNK)
                ut = u[:, cols]
                lo = phi6[:, cols]
                hi = phi_hi[:, cols]
                # u = clip(theta, 1e-6, 1-1e-6) (equivalent to the reference's
                # clip(clip(theta,0,1), 1e-6, 1-1e-6) up to f16 rounding)
                nc.vector.tensor_scalar(
                    ut, theta_sb[:, cols], U_LO, U_HI, ALU.max, ALU.min
                )
                # Horner: ((A*u + B)*u + C)*u + D with per-partition A..D
                nc.vector.tensor_scalar(lo, ut, cf(0), None, ALU.mult)
                nc.vector.scalar_tensor_tensor(lo, lo, cf(1), ut, ALU.add, ALU.mult)
                nc.vector.scalar_tensor_tensor(lo, lo, cf(2), ut, ALU.add, ALU.mult)
                nc.vector.tensor_scalar(lo, lo, cf(3), None, ALU.add)

                nc.vector.tensor_scalar(hi, ut, cf(4), None, ALU.mult)
                nc.vector.scalar_tensor_tensor(hi, hi, cf(5), ut, ALU.add, ALU.mult)
                nc.vector.scalar_tensor_tensor(hi, hi, cf(6), ut, ALU.add, ALU.mult)
                nc.vector.tensor_scalar(hi, hi, cf(7), None, ALU.add)

                # mask overwrites u (no longer needed); CopyPredicated wants an
                # integer mask; bitcast f16 1.0/0.0 (0x3C00/0x0).
                nc.vector.tensor_scalar(ut, ut, 0.5, None, ALU.is_ge)
                nc.vector.copy_predicated(lo, ut.bitcast(mybir.dt.uint16), hi)

            # ---- main streaming loop ----
            # phi chunks are emitted just-in-time inside the tile loop so the
            # in-order DVE queue doesn't stall tile 0 behind all of phi.
            tiles_per_phi = PHI_CHUNK // P
            for j in range(N_TILES):
                if j % tiles_per_phi == 0:
                    emit_phi_chunk(j // tiles_per_phi)
                rows = slice(j * P, (j + 1) * P)
                # f32 -> f16 cast during the DMA (SWDGE): halves the SBUF-side
                # bytes of the dominant x stream; ~5e-4 relative rounding on x.
                xt = xpool.tile([P, D], F16, tag="xt")
                nc.gpsimd.dma_start(out=xt, in_=x[rows, :])
                ot = opool.tile([P, D], F16)

                for c in range(D // CHUNK):
                    cols = slice(c * CHUNK, (c + 1) * CHUNK)
                    ps = ppool.tile([P, CHUNK], F32)
                    for s in range(CHUNK // MM_N):
                        nc.tensor.matmul(
                            ps[:, s * MM_N : (s + 1) * MM_N],
                            phi6[:, j * P : (j + 1) * P],
                            w_sb[:, c * CHUNK + s * MM_N : c * CHUNK + (s + 1) * MM_N],
                            start=True,
                            stop=False,
                        )
                    nc.vector.tensor_mul(out=ps, in0=ps, in1=xt[:, cols])
                    for s in range(CHUNK // MM_N):
                        nc.tensor.matmul(
                            ps[:, s * MM_N : (s + 1) * MM_N],
                            phi6[:, j * P : (j + 1) * P],
                            w_sb[
                                :,
                                D + c * CHUNK + s * MM_N : D
                                + c * CHUNK
                                + (s + 1) * MM_N,
                            ],
                            start=False,
                            stop=True,
                        )
                    # PSUM -> SBUF with f32 -> f16 cast
                    nc.scalar.copy(out=ot[:, cols], in_=ps)
                nc.scalar.dma_start(out=out[rows, :], in_=ot)
    nc.compile()
    return nc


_NC_CACHE = None


def _get_nc():
    global _NC_CACHE
    if _NC_CACHE is None:
        _NC_CACHE = _build_nc()
    return _NC_CACHE


def _make_in_maps(x, theta, Wa, ca, Wb, cb):
    x = np.ascontiguousarray(x, dtype=np.float32)
    theta = np.ascontiguousarray(theta, dtype=np.float32).reshape(-1)
    wab = np.empty((K + 1, 2 * D), dtype=np.float16)
    wab[:K, :D] = Wa.T.astype(np.float16)
    wab[K, :D] = ca.astype(np.float16)
    wab[:K, D:] = Wb.T.astype(np.float16)
    wab[K, D:] = cb.astype(np.float16)
    coef6 = np.zeros((K + 1, 8), dtype=np.float32)
    coef6[:K] = SPLINE_COEF
    coef6[K] = [0, 0, 0, 1, 0, 0, 0, 1]  # bias row: poly == 1.0
    in_maps = []
    for core in range(N_CORES):
        rows = slice(core * B_SHARD, (core + 1) * B_SHARD)
        thetab = np.broadcast_to(
            theta[rows][None, :].astype(np.float16), (K + 1, B_SHARD)
        ).copy()
        in_maps.append(
            {
                "x": np.ascontiguousarray(x[rows]),
                "thetab": thetab,
                "coefb": coef6,
                "wab": wab,
            }
        )
    return in_maps


def _run(inputs, trace=False, **kwargs):
    nc = _get_nc()
    in_maps = _make_in_maps(**inputs)
    res = run_bass_kernel_spmd(
        nc, in_maps, core_ids=list(range(N_CORES)), trace=trace, **kwargs
    )
    # Device computes/stores f16; widen to the reference's f32 during unshard.
    out = np.concatenate([r["out"] for r in res.results], axis=0).astype(np.float32)
    return out, res


def kernel(**inputs):
    out, _ = _run(inputs, trace=False)
    return out
